# revision 1
# baseline (speedup 1.0000x reference)
"""GCN encoder (3-layer GCNConv + BatchNorm + global_mean_pool) on 8 trn2 cores.

v3 strategy (edge partition by SOURCE core + piece-split ReduceScatter + fp8):
- Nodes split into 8 ranges of NDp=6272 (49 blocks x 128).  Each layer stores
  dinv-scaled post-relu activations  acheck = dinv * a  as an fp8(e3m4) table
  [NDp, H] in LOCAL DRAM (row = (rr%128)*NBLK + rr//128).
- Edges are processed on the core owning their SOURCE node: rows fetched from
  the local table with dma_gather (128B descriptors), aggregated per global
  dst block via one-hot matmuls (fp8 lhsT x f16 one-hot) into aggT [H, 128]
  PSUM, staged f16 into partial tables laid out so ReduceScatter chunk c =
  [128 feat, cols] slab for dst core c.  The RS is split in two column pieces:
  piece A's RS + readback + transform overlap piece B's aggregation.
- BN normalization folded algebraically: pre = dinv^2*(uhatT^T@(c1*W) +
  stilde(x)(c0@W) + rdinv(x)b), acheck = relu via one fused tensor_scalar.
  BN stats (weighted ones-matmuls on the f16 copy) AllReduce overlaps the
  next layer's gather.
- Layer 0 from host-precomputed norm-weighted vocab counts (structure-only):
  pre0 = u0 @ (embed@W0) + b0.  No gather, no collective.
- Pool over raw a(2) via sqrt(deg)-weighted batch one-hots; pool partials and
  layer-2 BN stats share one AllReduce [66, H].
"""
import os

import numpy as np

import concourse.bass as bass
import concourse.bacc as bacc
import concourse.tile as tile
from concourse import mybir
from concourse.bass_utils import run_bass_kernel_spmd
from concourse.masks import make_identity

N = 50000
E = 800000
H = 128
L = 3
V = 30
VP = 32              # padded vocab
G = 64
NC = 8
P = 128
NDp = 6272           # padded nodes per core (49 * 128)
NBLK = NDp // P      # 49
TR = NC * NDp        # 50176
NPAD = TR - N        # 176 pad nodes (all on core 7)
NGBLK = NC * NBLK    # 392 global dst blocks
BN_EPS = 1e-5
PIECES = ((0, 17), (17, 16), (33, 10), (43, 6))  # (j_start, n_blocks) per piece
GRS_P = ((17,), (16,), (10,), (6,))  # granule sizes per piece (per core range)

F32 = mybir.dt.float32
F16 = mybir.dt.float16
I16 = mybir.dt.int16
I32 = mybir.dt.int32
F8 = {"e3": mybir.dt.float8e3, "e4": mybir.dt.float8e4,
      "16": mybir.dt.float16}[os.environ.get("KF8", "16")]
TAB8 = F8 != mybir.dt.float16
SLABR = 128 + 2          # piece-A slab rows: 128 feat + 2 ridden stats rows


def _wrap16(flat):
    """dma_gather index layout: [128, n/16] int16, (p, s) -> flat[s*16 + p%16]."""
    n = flat.size
    w = flat.reshape(n // 16, 16).T.astype(np.int16)
    return np.ascontiguousarray(np.tile(w, (8, 1)))


def _pm(a):
    """node-local array [NDp] -> partition-major [128, NBLK]: (p, j) = a[j*128+p]."""
    return np.ascontiguousarray(a.reshape(NBLK, P).T)


def _block_order():
    """ORDER position -> global block id, grouped (piece, core, j)."""
    order = []
    for (j0, nj) in PIECES:
        for c in range(NC):
            for j in range(j0, j0 + nj):
                order.append(c * NBLK + j)
    return np.array(order, dtype=np.int64)


def _granules():
    """(piece, dst_core, j_start, gr, order_pos) per granule."""
    out = []
    pos = 0
    for pidx, (pj0, nj) in enumerate(PIECES):
        for c in range(NC):
            j = pj0
            for gr in GRS_P[pidx]:
                out.append((pidx, c, j, gr, pos))
                pos += gr
                j += gr
    return out


def _prep(x, edge_index, batch):
    """Host-side sharding/index prep (structure-only). Returns (sched, ins, pool)."""
    x = np.asarray(x).astype(np.int64)
    ei = np.asarray(edge_index).astype(np.int64)
    batch = np.asarray(batch).astype(np.int64)

    row = np.concatenate([ei[0], np.arange(N, dtype=np.int64)])
    col = np.concatenate([ei[1], np.arange(N, dtype=np.int64)])
    deg_r = np.bincount(col, minlength=N).astype(np.float64)

    deg = np.ones(TR)
    deg[:N] = deg_r
    dinv = 1.0 / np.sqrt(deg)
    # stilde incl the appended self-loop; pads behave as isolated self-nodes
    stilde = np.ones(TR)
    stilde[:N] = np.bincount(col, weights=dinv[row], minlength=N)

    # layer-0 norm-weighted vocab counts u0[d, v] (structure-only prep)
    w_e = dinv[row] * dinv[col]
    u0 = np.bincount(col * V + x[row], weights=w_e,
                     minlength=N * V).reshape(N, V)
    u0p = np.zeros((TR, VP), dtype=np.float32)
    u0p[:N, :V] = u0

    order = _block_order()
    rank = np.empty(NGBLK, dtype=np.int64)
    rank[order] = np.arange(NGBLK)

    # ---- per-core edge prep (partition by SOURCE core) ----
    # only the E raw edges: the appended self-loops are handled on-device by
    # adding the local acheck table to the RS output (uhatT += aT)
    erow, ecol = row[:E], col[:E]
    csrc = erow // NDp
    src_local = erow % NDp
    tsrc = (src_local % P) * NBLK + src_local // P    # acheck table row
    B = (ecol // NDp) * NBLK + (ecol % NDp) // P      # global dst block
    rB = rank[B]                                      # ORDER position
    q = (ecol % NDp) % P                              # dst pos within block

    counts = np.zeros((NC, NGBLK), dtype=np.int64)
    per_core = []
    for c in range(NC):
        m = csrc == c
        t_c, r_c, q_c = tsrc[m], rB[m], q[m]
        o = np.argsort(r_c, kind="stable")
        t_c, r_c, q_c = t_c[o], r_c[o], q_c[o]
        bnd = np.searchsorted(r_c, np.arange(NGBLK + 1))
        counts[c] = bnd[1:] - bnd[:-1]
        per_core.append((t_c, q_c, bnd))

    KB = np.maximum(-(-counts.max(axis=0) // P), 1)   # chunks per ORDER pos
    sched = tuple(int(k) for k in KB)

    ins = []
    for c in range(NC):
        t_c, q_c, bnd = per_core[c]
        idx_parts, cr_parts = [], []
        for pos in range(NGBLK):
            n = bnd[pos + 1] - bnd[pos]
            K = KB[pos]
            padv = t_c[bnd[pos]] if n > 0 else 0
            idx = np.full(K * P, padv, dtype=np.int64)
            crel = np.full(K * P, -1.0, dtype=np.float32)
            idx[:n] = t_c[bnd[pos]:bnd[pos + 1]]
            crel[:n] = q_c[bnd[pos]:bnd[pos + 1]]
            idx_parts.append(idx)
            cr_parts.append(crel.reshape(K, P).T)
        idx16 = _wrap16(np.concatenate(idx_parts))
        colrel = np.concatenate(cr_parts, axis=1).astype(np.float16)

        nodes = np.arange(c * NDp, (c + 1) * NDp)
        ins.append(dict(
            idx16=idx16,                                   # [128, CT*8]
            colrel=colrel,                                 # [128, CT]
            u0T=np.ascontiguousarray(
                u0p[nodes].T.astype(np.float16)),          # [32, NDp]
            stilde_row=stilde[nodes].astype(
                np.float16).reshape(1, NDp),
            rdinv_row=np.sqrt(deg[nodes]).astype(
                np.float16).reshape(1, NDp),
            dinvpm=_pm(dinv[nodes].astype(np.float32)),
            dinv2pm=_pm((dinv[nodes] ** 2).astype(np.float32)),
            sqdegpm32=_pm(np.sqrt(deg[nodes]).astype(np.float32)),
            sqdegpm16=_pm(np.sqrt(deg[nodes]).astype(np.float16)),
            degpm16=_pm(deg[nodes].astype(np.float16)),
            batchpm=_pm(np.where(nodes < N, batch[np.minimum(nodes, N - 1)],
                                 -1.0).astype(np.float32)),
        ))

    cntraw = np.bincount(batch, minlength=G).astype(np.float32)
    invcnt = (1.0 / np.maximum(cntraw, 1.0)).astype(np.float32)
    return sched, ins, (cntraw.reshape(1, G), invcnt.reshape(1, G))


def _build(sched):
    KB = sched
    CT = sum(KB)
    coff = np.concatenate([[0], np.cumsum(KB)])   # chunk offset per ORDER pos

    nc = bacc.Bacc("TRN2", target_bir_lowering=False, debug=False,
                   num_devices=NC)

    embed_t = nc.dram_tensor("embed", [V, H], F32, kind="ExternalInput")
    W_t = nc.dram_tensor("W", [L, H, H], F32, kind="ExternalInput")
    b_t = nc.dram_tensor("b", [L, H], F32, kind="ExternalInput")
    gamma_t = nc.dram_tensor("gamma", [L, H], F32, kind="ExternalInput")
    beta_t = nc.dram_tensor("beta", [L, H], F32, kind="ExternalInput")
    idx16_t = nc.dram_tensor("idx16", [128, CT * 8], I16, kind="ExternalInput")
    colrel_t = nc.dram_tensor("colrel", [128, CT], F16, kind="ExternalInput")
    u0T_t = nc.dram_tensor("u0T", [VP, NDp], F16, kind="ExternalInput")
    stilde_t = nc.dram_tensor("stilde_row", [1, NDp], F16, kind="ExternalInput")
    rdinv_t = nc.dram_tensor("rdinv_row", [1, NDp], F16, kind="ExternalInput")
    dinvpm_t = nc.dram_tensor("dinvpm", [128, NBLK], F32, kind="ExternalInput")
    dinv2pm_t = nc.dram_tensor("dinv2pm", [128, NBLK], F32, kind="ExternalInput")
    sqdegpm32_t = nc.dram_tensor("sqdegpm32", [128, NBLK], F32, kind="ExternalInput")
    sqdegpm16_t = nc.dram_tensor("sqdegpm16", [128, NBLK], F16, kind="ExternalInput")
    degpm16_t = nc.dram_tensor("degpm16", [128, NBLK], F16, kind="ExternalInput")
    batchpm_t = nc.dram_tensor("batchpm", [128, NBLK], F32, kind="ExternalInput")
    cntrow_t = nc.dram_tensor("cntrow", [1, G], F32, kind="ExternalInput")
    invcntrow_t = nc.dram_tensor("invcntrow", [1, G], F32, kind="ExternalInput")
    out_t = nc.dram_tensor("out", [G, H], F32, kind="ExternalOutput")

    rg = [list(range(NC))]
    AF = mybir.ActivationFunctionType
    OP = mybir.AluOpType
    GRAN = _granules()

    with tile.TileContext(nc) as tc:
        with tc.tile_pool(name="big", bufs=1) as big, \
             tc.tile_pool(name="sm", bufs=1) as sm, \
             tc.tile_pool(name="smd", bufs=2) as smd, \
             tc.tile_pool(name="gpool", bufs=3) as gpool, \
             tc.tile_pool(name="spool", bufs=2) as spool, \
             tc.tile_pool(name="stg", bufs=2) as stgp, \
             tc.tile_pool(name="ps", bufs=2, space="PSUM") as ps, \
             tc.tile_pool(name="psacc", bufs=2, space="PSUM") as psacc, \
             tc.tile_pool(name="dram", bufs=1, space="DRAM") as dram:

            # ---------- constants / inputs ----------
            ident = sm.tile([P, P], F32)
            make_identity(nc, ident[:])
            ident16 = sm.tile([P, P], F16)
            nc.vector.tensor_copy(ident16[:], ident[:])
            iota_i = sm.tile([P, P], I32)
            nc.gpsimd.iota(iota_i[:], pattern=[[1, P]], base=0,
                           channel_multiplier=0)
            iota16 = sm.tile([P, P], F16)
            nc.vector.tensor_copy(iota16[:], iota_i[:])
            iota64_i = sm.tile([P, G], I32)
            nc.gpsimd.iota(iota64_i[:], pattern=[[1, G]], base=0,
                           channel_multiplier=0)
            iota64_f = sm.tile([P, G], F32)
            nc.vector.tensor_copy(iota64_f[:], iota64_i[:])

            # load order: L0's dependencies first (embed/W0/u0T/cols), the
            # big gather index tables after (first needed ~50us in)
            embsb = sm.tile([V, H], F32)
            nc.sync.dma_start(embsb[:], embed_t.ap())
            # u0T lives in the S-pool ring: only needed during layer 0, its
            # buffer is recycled by layer-1 granule one-hots afterwards
            u0T_sb = spool.tile([VP, NDp], F16, tag="S", name="u0T")
            nc.sync.dma_start(u0T_sb[:], u0T_t.ap())
            dinvpm_sb = sm.tile([128, NBLK], F32)
            nc.sync.dma_start(dinvpm_sb[:], dinvpm_t.ap())
            sqdeg16_sb = sm.tile([128, NBLK], F16)
            nc.sync.dma_start(sqdeg16_sb[:], sqdegpm16_t.ap())
            deg16_sb = sm.tile([128, NBLK], F16)
            nc.sync.dma_start(deg16_sb[:], degpm16_t.ap())

            Wsb = [sm.tile([H, H], F32, name=f"W{l}") for l in range(L)]
            bcol = [sm.tile([H, 1], F32, name=f"b{l}") for l in range(L)]
            gcol = [sm.tile([H, 1], F32, name=f"g{l}") for l in range(L)]
            betacol = [sm.tile([H, 1], F32, name=f"be{l}") for l in range(L)]
            for l in range(L):
                nc.sync.dma_start(Wsb[l][:], W_t.ap()[l])
                nc.sync.dma_start(bcol[l][:], b_t.ap()[l, :, None])
                nc.sync.dma_start(gcol[l][:], gamma_t.ap()[l, :, None])
                nc.sync.dma_start(betacol[l][:], beta_t.ap()[l, :, None])
            b32row = [sm.tile([1, H], F32, name=f"b32r{l}") for l in range(L)]
            b16row = [sm.tile([1, H], F16, name=f"b16r{l}") for l in range(L)]
            for l in range(L):
                nc.sync.dma_start(b32row[l][:], b_t.ap()[l:l + 1, :])
                nc.vector.tensor_copy(b16row[l][:], b32row[l][:])
            idx_sb = sm.tile([128, CT * 8], I16)
            nc.sync.dma_start(idx_sb[:], idx16_t.ap())
            colrel_sb = sm.tile([128, CT], F16)
            nc.sync.dma_start(colrel_sb[:], colrel_t.ap())
            stilde_sb = sm.tile([1, NDp], F16)
            nc.sync.dma_start(stilde_sb[:], stilde_t.ap())
            rdinv_sb = sm.tile([1, NDp], F16)
            nc.sync.dma_start(rdinv_sb[:], rdinv_t.ap())
            dinv2pm_sb = sm.tile([128, NBLK], F32)
            nc.sync.dma_start(dinv2pm_sb[:], dinv2pm_t.ap())
            sqdeg32_sb = sm.tile([128, NBLK], F32)
            nc.sync.dma_start(sqdeg32_sb[:], sqdegpm32_t.ap())
            batchpm_sb = sm.tile([128, NBLK], F32)
            nc.sync.dma_start(batchpm_sb[:], batchpm_t.ap())
            cntbc = sm.tile([128, G], F32)
            nc.sync.dma_start(cntbc[:], bass.AP(tensor=cntrow_t, offset=0,
                                                ap=[[0, 128], [1, G]]))
            invcntbc = sm.tile([128, G], F32)
            nc.sync.dma_start(invcntbc[:], bass.AP(tensor=invcntrow_t,
                                                   offset=0,
                                                   ap=[[0, 128], [1, G]]))
            eps_sb = sm.tile([H, 1], F32)
            nc.vector.memset(eps_sb[:], BN_EPS)
            ones_row = sm.tile([1, NDp], F16)
            nc.vector.memset(ones_row[:], 1.0)

            # ---------- persistent big tiles ----------
            A16 = [big.tile([128, NBLK, H], F16, name=f"A16_{i}")
                   for i in range(2)]
            A8 = ([big.tile([128, NBLK, H], F8, name=f"A8_{i}")
                   for i in range(2)] if TAB8 else A16)
            uhatT = big.tile([128, NDp], F16)
            aT = big.tile([128, NDp], F16)      # transposed acheck (self-add)

            # ---------- DRAM scratch ----------
            atab_d = [dram.tile([NDp, H], F8, name=f"atab{l}")
                      for l in range(L - 1)]
            # piece-A slabs carry 2 extra rows: the previous layer's BN stat
            # sums ride the ReduceScatter (replicated per slab -> all-reduced)
            pagg_d = [[dram.tile(
                [NC * (SLABR if pi == 0 else 128), nj * P], F16,
                name=f"pagg{i}_{pi}") for pi, (_, nj) in enumerate(PIECES)]
                for i in range(2)]
            rsout_d = [[dram.tile(
                [(SLABR if pi == 0 else 128), nj * P], F16,
                name=f"rsout{i}_{pi}") for pi, (_, nj) in enumerate(PIECES)]
                for i in range(2)]
            pool_i = dram.tile([G + 2, H], F16)
            pool_o = dram.tile([NC, G + 2, H], F16, addr_space="Shared")

            # ---------- EW0 = (embed @ W0) as f16 [VP, H] ----------
            trp = ps.tile([P, P], F32, tag="tr", space="PSUM", bufs=1)
            nc.tensor.transpose(out=trp[:, 0:V], in_=embsb[:],
                                identity=ident[0:V, 0:V])
            embT = sm.tile([128, VP], F32)
            nc.vector.memset(embT[:], 0.0)
            nc.vector.tensor_copy(embT[:, 0:V], trp[:, 0:V])
            ew0p = ps.tile([P, H], F32, tag="mp", space="PSUM", bufs=2)
            nc.tensor.matmul(out=ew0p[0:VP, :], lhsT=embT[:], rhs=Wsb[0][:],
                             start=True, stop=True)
            EW0 = sm.tile([VP, H], F16)
            nc.vector.tensor_copy(EW0[:], ew0p[0:VP, :])

            poolp = psacc.tile([G, H], F32, tag="pool", space="PSUM", bufs=1)

            def transform(l, j_lo, j_hi, lhsT_fn, rhs_main, extra_rank1,
                          scale_col_sb, SA, SSA):
                """blocks j_lo..j_hi-1: pre-psum -> acheck + stats (+pool).
                Two sweeps so the in-order PE queue never stalls on the
                DVE-produced acheck: (1) pre-matmuls + fused scale/relu,
                (2) stats/pool matmuls reading A16."""
                Ai = A16[l % 2]
                for j in range(j_lo, j_hi):
                    pre = ps.tile([P, H], F32, tag="mp", space="PSUM", bufs=2)
                    nterm = 1 + len(extra_rank1)
                    nc.tensor.matmul(out=pre[:], lhsT=lhsT_fn(j),
                                     rhs=rhs_main, start=True,
                                     stop=(nterm == 1))
                    for t, (rsb, rrow) in enumerate(extra_rank1):
                        nc.tensor.matmul(out=pre[:],
                                         lhsT=rsb[0:1, j * P:(j + 1) * P],
                                         rhs=rrow, start=False,
                                         stop=(t == nterm - 2))
                    # acheck = relu(scale * pre)  (fused mult + max 0)
                    nc.vector.tensor_scalar(
                        out=Ai[:, j, :], in0=pre[:],
                        scalar1=scale_col_sb[:, j:j + 1], scalar2=0.0,
                        op0=OP.mult, op1=OP.max)
                    if TAB8 and l < L - 1:
                        nc.scalar.activation(out=A8[l % 2][:, j, :],
                                             in_=Ai[:, j, :], func=AF.Copy)
                for j in range(j_lo, j_hi):
                    sq = smd.tile([P, H], F16, tag="sq", bufs=3)
                    nc.vector.tensor_mul(sq[:], Ai[:, j, :], Ai[:, j, :])
                    nc.tensor.matmul(out=SA, lhsT=sqdeg16_sb[:, j:j + 1],
                                     rhs=Ai[:, j, :], start=(j == 0),
                                     stop=(j == NBLK - 1))
                    nc.tensor.matmul(out=SSA, lhsT=deg16_sb[:, j:j + 1],
                                     rhs=sq[:], start=(j == 0),
                                     stop=(j == NBLK - 1))
                    if l == L - 1:
                        Sb = smd.tile([P, G], F16, tag="Sb", bufs=3)
                        nc.vector.tensor_scalar(
                            out=Sb[:], in0=iota64_f[:],
                            scalar1=batchpm_sb[:, j:j + 1],
                            scalar2=sqdeg32_sb[:, j:j + 1],
                            op0=OP.is_equal, op1=OP.mult)
                        nc.tensor.matmul(out=poolp[:], lhsT=Sb[:],
                                         rhs=Ai[:, j, :], start=(j == 0),
                                         stop=(j == NBLK - 1))

            def bn_cols(l, mvT, apad_col):
                """mvT [H, 2] f32 (AR'd sums) -> c1, c0 columns [H, 1].
                apad_col = value of a at pad nodes this layer (isolated-node
                recursion), used for the analytic pad correction."""
                # mvT rows arrive pre-scaled by 1/N (f16 overflow safety)
                rbs = smd.tile([H, 1], F32, tag="rbs")
                nc.vector.tensor_scalar(out=rbs[:], in0=apad_col[:],
                                        scalar1=float(NPAD) / N, scalar2=None,
                                        op0=OP.mult)
                mu = smd.tile([H, 1], F32, tag="mu")
                nc.vector.tensor_sub(mu[:], mvT[:, 0:1], rbs[:])
                rb2 = smd.tile([H, 1], F32, tag="rb2")
                nc.vector.tensor_mul(rb2[:], apad_col[:], apad_col[:])
                nc.vector.tensor_scalar(out=rb2[:], in0=rb2[:],
                                        scalar1=float(NPAD) / N, scalar2=None,
                                        op0=OP.mult)
                e2 = smd.tile([H, 1], F32, tag="e2")
                nc.vector.tensor_sub(e2[:], mvT[:, 1:2], rb2[:])
                var = smd.tile([H, 1], F32, tag="var")
                nc.vector.tensor_mul(var[:], mu[:], mu[:])
                nc.vector.tensor_sub(var[:], e2[:], var[:])
                sd = smd.tile([H, 1], F32, tag="sd")
                nc.scalar.activation(out=sd[:], in_=var[:], func=AF.Sqrt,
                                     bias=eps_sb[:], scale=1.0)
                rstd = smd.tile([H, 1], F32, tag="rstd")
                nc.vector.reciprocal(rstd[:], sd[:])
                c1 = smd.tile([H, 1], F32, tag=f"c1_{l}", bufs=1)
                nc.vector.tensor_mul(c1[:], gcol[l][:], rstd[:])
                c0 = smd.tile([H, 1], F32, tag=f"c0_{l}", bufs=1)
                nc.vector.tensor_mul(c0[:], mu[:], c1[:])
                nc.vector.tensor_sub(c0[:], betacol[l][:], c0[:])
                return c1, c0

            def emit_stats_ride(l, SA, SSA):
                """Write layer-l stat sums (f16) into every slab's stats rows
                of the NEXT layer's piece-A partial table; RS-A all-reduces
                them as a side effect."""
                sa16 = smd.tile([1, H], F16, tag="sasb")
                ss16 = smd.tile([1, H], F16, tag="sssb")
                nc.scalar.activation(out=sa16[:], in_=SA, func=AF.Copy,
                                     scale=1.0 / N)
                nc.scalar.activation(out=ss16[:], in_=SSA, func=AF.Copy,
                                     scale=1.0 / N)
                pa = pagg_d[l][0]
                for c in range(NC):
                    r0 = c * SLABR + 128
                    nc.sync.dma_start(pa[:][r0:r0 + 1, 0:H], sa16[:])
                    nc.sync.dma_start(pa[:][r0 + 1:r0 + 2, 0:H], ss16[:])

            def atab_write(l, j_lo, j_hi):
                src = A8[l % 2][:, j_lo:j_hi, :]
                dst = atab_d[l][:].rearrange(
                    "(p j) h -> p j h", p=128)[:, j_lo:j_hi, :]
                nc.sync.dma_start(dst, src)

            def transpose_some(l, js):
                """aT[:, j*128:(j+1)*128] = A16[l%2][:, j, :]^T (self-add)."""
                for j in js:
                    t16 = ps.tile([P, H], F16, tag="mp", space="PSUM", bufs=2)
                    nc.tensor.transpose(out=t16[:], in_=A16[l % 2][:, j, :],
                                        identity=ident16[:])
                    nc.scalar.activation(out=aT[:, j * P:(j + 1) * P],
                                         in_=t16[:], func=AF.Copy)

            def transpose_pass(l):
                transpose_some(l, range(NBLK))

            apad = [sm.tile([H, 1], F32, name=f"apad{l}") for l in range(L)]
            nc.scalar.activation(out=apad[0][:], in_=bcol[0][:], func=AF.Relu)

            def apad_next(l, c1, c0):
                """apad[l] = relu((c1*apad[l-1] + c0) @ W[l] + b[l])."""
                hp = smd.tile([H, 1], F32, tag="hp")
                nc.vector.tensor_mul(hp[:], c1[:], apad[l - 1][:])
                nc.vector.tensor_add(hp[:], hp[:], c0[:])
                pcp = ps.tile([P, P], F32, tag="tr", space="PSUM", bufs=1)
                nc.tensor.matmul(out=pcp[0:H, 0:1], lhsT=Wsb[l][:],
                                 rhs=hp[:], start=True, stop=True)
                nc.scalar.activation(out=apad[l][:], in_=pcp[0:H, 0:1],
                                     func=AF.Relu, bias=bcol[l][:], scale=1.0)

            PH = {"l0": 0, "u1": 1, "t1": 2, "t2": 3, "s1": 4, "w1": 5,
                  "u1b": 6, "full": 9}[os.environ.get("KPHASE", "full")]

            def dump(ap):
                dbg = sm.tile([G, H], F32, name="dbg")
                nc.vector.tensor_copy(dbg[:], ap)
                nc.sync.dma_start(out_t.ap(), dbg[:])

            # ================= layer 0 =================
            SA0t = psacc.tile([1, H], F32, tag="SA", space="PSUM", bufs=1)
            SSA0t = psacc.tile([1, H], F32, tag="SSA", space="PSUM", bufs=1)
            SA0 = SA0t[:]
            SSA0 = SSA0t[:]
            transform(0, 0, NBLK, lambda j: u0T_sb[:, j * P:(j + 1) * P],
                      EW0[:], [(ones_row, b16row[0][:])], dinvpm_sb,
                      SA0, SSA0)
            atab_write(0, 0, NBLK)
            emit_stats_ride(0, SA0, SSA0)
            if PH == 0:
                dump(A16[0][0:G, 0, :])

            # ================= layers 1..2 =================
            SA = SSA = None
            NLAY = {0: 1, 1: 2, 2: 2, 3: L, 4: L, 5: 2, 6: 2, 9: L}[PH]
            for l in range(1, NLAY):
                li = l - 1          # pagg/rsout/atab parity index

                def granules_of(pidx, pj0):
                    # distribute this piece's aT transposes (of layer l-1)
                    # across its granules so they hide inside the sweep and
                    # finish before this piece's readback self-add
                    nj_p = PIECES[pidx][1]
                    tq = list(range(pj0, pj0 + nj_p))
                    gi = 0
                    for (gp, cB, j0, gr, pos0) in GRAN:
                        if gp != pidx:
                            continue
                        kg = sum(KB[pos0 + t] for t in range(gr))
                        o8 = coff[pos0] * 8
                        oc = coff[pos0]
                        gt = gpool.tile([128, kg, H], F8, tag="gt")
                        nc.gpsimd.dma_gather(
                            out_ap=gt[:], in_ap=atab_d[l - 1][:],
                            idxs_ap=idx_sb[:, o8:o8 + kg * 8],
                            num_idxs=kg * P, num_idxs_reg=kg * P,
                            elem_size=H, single_packet=False, queue_num=0)
                        S = spool.tile([128, kg, H], F16, tag="S")
                        cr = colrel_sb[:, oc:oc + kg]
                        cr3 = bass.AP(tensor=colrel_sb.tensor,
                                      offset=cr.offset,
                                      ap=[cr.ap[0], cr.ap[1], [0, P]])
                        io3 = bass.AP(tensor=iota16.tensor,
                                      offset=iota16[:].offset,
                                      ap=[iota16[:].ap[0], [0, kg], [1, P]])
                        nc.vector.tensor_tensor(out=S[:], in0=cr3, in1=io3,
                                                op=OP.is_equal)
                        stg = stgp.tile([128, gr * P], F16, tag="stg")
                        ko = 0
                        slabr = SLABR if pidx == 0 else 128
                        for t in range(gr):
                            aggp = psacc.tile([H, P], F32, tag="agg",
                                              space="PSUM", bufs=2)
                            for i in range(KB[pos0 + t]):
                                nc.tensor.matmul(out=aggp[:],
                                                 lhsT=gt[:, ko + i, :],
                                                 rhs=S[:, ko + i, :],
                                                 start=(i == 0),
                                                 stop=(i == KB[pos0 + t] - 1))
                            nc.scalar.activation(
                                out=stg[:, t * P:(t + 1) * P],
                                in_=aggp[:], func=AF.Copy)
                            ko += KB[pos0 + t]
                        dst = pagg_d[li][pidx][:][
                            cB * slabr:cB * slabr + 128,
                            (j0 - pj0) * P:(j0 - pj0 + gr) * P]
                        nc.sync.dma_start(dst, stg[:])
                        gi += 1
                        hi = (nj_p * gi) // NC
                        lo = (nj_p * (gi - 1)) // NC
                        if hi > lo:
                            transpose_some(l - 1, tq[lo:hi])
                    nc.gpsimd.collective_compute(
                        "ReduceScatter", OP.add, replica_groups=rg,
                        ins=[pagg_d[li][pidx].opt()],
                        outs=[rsout_d[li][pidx].opt()])

                # piece A aggregation + RS first (stats rows ride along);
                # BN prep can then overlap the remaining pieces' aggregation
                granules_of(0, PIECES[0][0])

                # ---- BN coefficients from stats ridden on RS-A ----
                ars = smd.tile([2, H], F16, tag="ars")
                nc.sync.dma_start(ars[:], rsout_d[li][0][:][128:130, 0:H])
                ars32 = smd.tile([2, H], F32, tag="ars32")
                nc.vector.tensor_copy(ars32[:], ars[:])
                mvp = ps.tile([P, P], F32, tag="tr", space="PSUM", bufs=1)
                nc.tensor.transpose(out=mvp[0:H, 0:2], in_=ars32[:],
                                    identity=ident[0:2, 0:2])
                mvT = smd.tile([H, 2], F32, tag="mvT")
                nc.vector.tensor_copy(mvT[:], mvp[0:H, 0:2])
                if PH == 4 and l == L - 1:
                    dbg = sm.tile([G, H], F32, name="dbg")
                    nc.vector.memset(dbg[:], 0.0)
                    nc.vector.tensor_copy(dbg[0:2, :], ars32[:])
                    nc.sync.dma_start(out_t.ap(), dbg[:])
                    break
                c1, c0 = bn_cols(l - 1, mvT, apad[l - 1])
                apad_next(l, c1, c0)
                Wc1 = smd.tile([H, H], F16, tag="Wc1")
                nc.vector.tensor_scalar(out=Wc1[:], in0=Wsb[l][:],
                                        scalar1=c1[:], scalar2=None,
                                        op0=OP.mult)
                c0wp = ps.tile([P, P], F32, tag="tr", space="PSUM", bufs=1)
                nc.tensor.matmul(out=c0wp[0:1, 0:H], lhsT=c0[:],
                                 rhs=Wsb[l][:], start=True, stop=True)
                c0Wrow = smd.tile([1, H], F16, tag="c0W")
                nc.vector.tensor_copy(c0Wrow[:], c0wp[0:1, 0:H])

                # remaining pieces' aggregation + RS
                for pidx in range(1, len(PIECES)):
                    granules_of(pidx, PIECES[pidx][0])

                # ---- per-piece readback + transform (+ table write) ----
                if PH not in (1, 6):
                    SAt = psacc.tile([1, H], F32, tag="SA", space="PSUM",
                                     bufs=1)
                    SSAt = psacc.tile([1, H], F32, tag="SSA", space="PSUM",
                                      bufs=1)
                    SA = SAt[:]
                    SSA = SSAt[:]
                for pidx, (pj0, nj) in enumerate(PIECES):
                    nc.sync.dma_start(
                        uhatT[:, pj0 * P:(pj0 + nj) * P],
                        rsout_d[li][pidx][:][0:128, :])
                    # self-loop contribution: uhat += acheck(l-1) transposed
                    nc.vector.tensor_add(
                        uhatT[:, pj0 * P:(pj0 + nj) * P],
                        uhatT[:, pj0 * P:(pj0 + nj) * P],
                        aT[:, pj0 * P:(pj0 + nj) * P])
                    if PH in (1, 6):
                        continue
                    transform(l, pj0, pj0 + nj,
                              lambda j: uhatT[:, j * P:(j + 1) * P], Wc1[:],
                              [(stilde_sb, c0Wrow[:]),
                               (rdinv_sb, b16row[l][:])],
                              dinv2pm_sb, SA, SSA)
                    if l < L - 1:
                        atab_write(l, pj0, pj0 + nj)
                if PH == 1:
                    dump(uhatT[0:G, 0:H])
                elif PH == 6:
                    dj = int(os.environ.get("KDUMPJ", str(PIECES[1][0])))
                    dump(uhatT[0:G, dj * P:dj * P + H])
                elif PH == 2 and l == 1:
                    dj = int(os.environ.get("KDUMPJ", "0"))
                    dump(A16[1][0:G, dj, :])
                elif PH == 3 and l == L - 1:
                    dump(A16[(L - 1) % 2][0:G, 0, :])
                if PH in (3, 4, 5, 9) and l < L - 1:
                    emit_stats_ride(l, SA, SSA)
                if PH == 5 and l == 1:
                    for c in range(NC):
                        r0 = c * SLABR + 128
                        row16 = smd.tile([2, H], F16, tag="dbg16", bufs=8)
                        nc.sync.dma_start(
                            row16[:], pagg_d[1][0][:][r0:r0 + 2, 0:H])
                        row32 = smd.tile([2, H], F32, tag="dbg32", bufs=8)
                        nc.vector.tensor_copy(row32[:], row16[:])
                        nc.sync.dma_start(out_t.ap()[2 * c:2 * c + 2, :],
                                          row32[:])

            if PH == 9:
                # ================= pooling tail =================
                packp = sm.tile([G, H], F16)
                nc.vector.tensor_copy(packp[:], poolp[:])
                sa_sb = smd.tile([1, H], F16, tag="sasb")
                ss_sb = smd.tile([1, H], F16, tag="sssb")
                nc.scalar.activation(out=sa_sb[:], in_=SA, func=AF.Copy,
                                     scale=1.0 / N)
                nc.scalar.activation(out=ss_sb[:], in_=SSA, func=AF.Copy,
                                     scale=1.0 / N)
                nc.sync.dma_start(pool_i[:][0:G, :], packp[:])
                nc.sync.dma_start(pool_i[:][G:G + 1, :], sa_sb[:])
                nc.sync.dma_start(pool_i[:][G + 1:G + 2, :], ss_sb[:])
                nc.gpsimd.collective_compute(
                    "AllGather", OP.bypass, replica_groups=rg,
                    ins=[pool_i.opt()], outs=[pool_o.opt()])
                parf = sm.tile([G + 2, NC * H], F16)
                nc.sync.dma_start(
                    parf[:], bass.AP(tensor=pool_o.tensor,
                                     offset=pool_o[:].offset,
                                     ap=[[H, G + 2], [(G + 2) * H, NC], [1, H]]))
                for s in (4, 2, 1):
                    for k in range(s):
                        nc.vector.tensor_add(
                            parf[:, k * H:(k + 1) * H],
                            parf[:, k * H:(k + 1) * H],
                            parf[:, (k + s) * H:(k + s + 1) * H])
                ptp = ps.tile([P, P], F16, tag="tr", space="PSUM", bufs=1)
                nc.tensor.transpose(out=ptp[0:H, 0:G + 2], in_=parf[:, 0:H],
                                    identity=ident16[0:G + 2, 0:G + 2])
                parT = sm.tile([H, G + 2], F32)
                nc.vector.tensor_copy(parT[:], ptp[0:H, 0:G + 2])
                c1f, c0f = bn_cols(L - 1, parT[:, G:G + 2], apad[L - 1])
                # outT[c,g] = (c1[c]*poolT + c0[c]*cnt[g]) * invcnt[g]
                t2 = sm.tile([H, G], F32)
                nc.vector.tensor_tensor(out=t2[:], in0=parT[:, 0:G],
                                        in1=invcntbc[:], op=OP.mult)
                nc.vector.tensor_scalar(out=t2[:], in0=t2[:], scalar1=c1f[:],
                                        scalar2=c0f[:], op0=OP.mult,
                                        op1=OP.add)
                fint = ps.tile([P, P], F32, tag="tr", space="PSUM", bufs=1)
                nc.tensor.transpose(out=fint[:G, :], in_=t2[:], identity=ident[:])
                outsb = sm.tile([G, H], F32)
                nc.vector.tensor_copy(outsb[:], fint[:G, :])
                nc.sync.dma_start(out_t.ap(), outsb[:])

    nc.compile()
    return nc


_NC_CACHE = {}


def _get_nc(sched):
    key = (sched, os.environ.get("KF8", "e3"))
    if key not in _NC_CACHE:
        _NC_CACHE[key] = _build(sched)
    return _NC_CACHE[key]


def run(x, edge_index, batch, embed, W, b, gamma, beta, trace=False):
    sched, per_core, (cntrow, invcntrow) = _prep(x, edge_index, batch)
    nc = _get_nc(sched)
    shared = dict(
        embed=np.ascontiguousarray(np.asarray(embed, dtype=np.float32)),
        W=np.ascontiguousarray(np.asarray(W, dtype=np.float32)),
        b=np.ascontiguousarray(np.asarray(b, dtype=np.float32)),
        gamma=np.ascontiguousarray(np.asarray(gamma, dtype=np.float32)),
        beta=np.ascontiguousarray(np.asarray(beta, dtype=np.float32)),
        cntrow=cntrow,
        invcntrow=invcntrow,
    )
    in_maps = [{**shared, **per_core[c]} for c in range(NC)]
    res = run_bass_kernel_spmd(nc, in_maps, core_ids=list(range(NC)),
                               trace=trace)
    return res.results[0]["out"], res


def kernel(x, edge_index, batch, embed, W, b, gamma, beta):
    out, _ = run(x, edge_index, batch, embed, W, b, gamma, beta)
    return out



# revision 17
# speedup vs baseline: 1.1493x; 1.1493x over previous
"""GCN encoder (3-layer GCNConv + BatchNorm + global_mean_pool) on 8 trn2 cores.

v4 strategy (equalized packed chunks; evolved from v3):
- Nodes split into 8 ranges of NDp=6272 (49 blocks x 128).  Each layer stores
  dinv-scaled post-relu activations  acheck = dinv * a  as an f16 table
  [NDp, H] in LOCAL DRAM (row = (rr%128)*NBLK + rr//128).
- Edges are processed on the core owning their SOURCE node.  Per (core, dst
  block) edge counts are EQUALIZED to the max across cores (pad edges carry
  colrel=-1), so all cores share one chunk/segment schedule.  Blocks are
  packed contiguously into 128-edge chunks (no per-block ceil-to-128), one
  one-hot S[128,128] per chunk built by a 4x-mode tensor_scalar(is_equal),
  and per (chunk, block) segments aggregated by partition-sliced matmuls
  into 4-block [H,512] PSUM groups (one ACT copy per group).
- Partials staged f16 into per-piece tables laid out so ReduceScatter chunk
  c = [128 feat, cols] slab for dst core c; piece A's RS + readback +
  transform overlap the remaining pieces' aggregation.
- BN normalization folded algebraically: pre = dinv^2*(uhatT^T@(c1*W) +
  stilde(x)(c0@W) + rdinv(x)b), acheck = relu via one fused tensor_scalar.
  BN stat sums ride the next layer's RS-A slabs (2 extra rows).
- Layer 0 from host-precomputed norm-weighted vocab counts (structure-only):
  pre0 = u0 @ (embed@W0) + b0.  No gather, no collective.
- Pool over raw a(2) via sqrt(deg)-weighted batch one-hots; pool partials and
  layer-2 BN stats share one AllGather [66, H].
"""
import os

import numpy as np

import concourse.bass as bass
import concourse.bacc as bacc
import concourse.tile as tile
from concourse import mybir
from concourse.bass_utils import run_bass_kernel_spmd
from concourse.masks import make_identity

N = 50000
E = 800000
H = 128
L = 3
V = 30
VP = 32              # padded vocab
G = 64
NC = 8
P = 128
NDp = 6272           # padded nodes per core (49 * 128)
NBLK = NDp // P      # 49
TR = NC * NDp        # 50176
NPAD = TR - N        # 176 pad nodes (all on core 7)
NGBLK = NC * NBLK    # 392 global dst blocks
BN_EPS = 1e-5
PIECES = ((0, 17), (17, 16), (33, 10), (43, 6))  # (j_start, n_blocks) per piece

F32 = mybir.dt.float32
F16 = mybir.dt.float16
I16 = mybir.dt.int16
I32 = mybir.dt.int32
F8 = F16
SLABR = 128 + 2          # piece-A slab rows: 128 feat + 2 ridden stats rows


def _wrap16(flat):
    """dma_gather index layout: [128, n/16] int16, (p, s) -> flat[s*16 + p%16]."""
    n = flat.size
    w = flat.reshape(n // 16, 16).T.astype(np.int16)
    return np.ascontiguousarray(np.tile(w, (8, 1)))


def _pm(a):
    """node-local array [NDp] -> partition-major [128, NBLK]: (p, j) = a[j*128+p]."""
    return np.ascontiguousarray(a.reshape(NBLK, P).T)


def _block_order():
    """ORDER position -> global block id, grouped (piece, core, j)."""
    order = []
    for (j0, nj) in PIECES:
        for c in range(NC):
            for j in range(j0, j0 + nj):
                order.append(c * NBLK + j)
    return np.array(order, dtype=np.int64)


def _granules():
    """(piece, dst_core, j_start, gr, order_pos0) per granule (one per
    (piece, dst core): gr = piece block count)."""
    out = []
    pos = 0
    for pidx, (pj0, nj) in enumerate(PIECES):
        for c in range(NC):
            out.append((pidx, c, pj0, nj, pos))
            pos += nj
    return out


def _schedule(cnt_eq):
    """Shared per-granule chunk/segment schedule.

    Blocks are packed contiguously into 128-edge chunks; a chunk may host at
    most TWO blocks (the rare 3rd forces a chunk break).  Each (chunk, block)
    segment is tagged with its one-hot column slot (0 = chunk's first block,
    1 = second), so aggregation matmuls always use the full 128 partitions
    and select the block via an rhs column slice of a [128, 2*P] one-hot.

    Returns list per granule of (kg, nsl, segs); segs = [(k, jj, slot, p0,
    p1, first, last)] with jj the block offset inside the granule and
    [p0, p1) the chunk partition rows of the segment; nsl[k] = slots used."""
    out = []
    for (pidx, cB, j0, gr, pos0) in _granules():
        segs = []
        chunk_blocks = {}
        off = 0
        for jj in range(gr):
            n = int(cnt_eq[pos0 + jj])
            k = off // P
            if off % P != 0 and len(chunk_blocks.get(k, [])) >= 2:
                off = (k + 1) * P
            first = True
            while True:
                k, p0 = divmod(off, P)
                bl = chunk_blocks.setdefault(k, [])
                if jj not in bl:
                    bl.append(jj)
                take = min(P - p0, n)
                n -= take
                segs.append((k, jj, bl.index(jj), p0, p0 + take,
                             first, n == 0))
                off += take
                first = False
                if n == 0:
                    break
        kg = -(-off // P)
        nsl = [len(chunk_blocks.get(k, [1])) for k in range(kg)]
        out.append((kg, nsl, segs))
    return out


def _prep(x, edge_index, batch):
    """Host-side sharding/index prep (structure-only). Returns (sched, ins, pool)."""
    x = np.asarray(x).astype(np.int64)
    ei = np.asarray(edge_index).astype(np.int64)
    batch = np.asarray(batch).astype(np.int64)

    row = np.concatenate([ei[0], np.arange(N, dtype=np.int64)])
    col = np.concatenate([ei[1], np.arange(N, dtype=np.int64)])
    deg_r = np.bincount(col, minlength=N).astype(np.float64)

    deg = np.ones(TR)
    deg[:N] = deg_r
    dinv = 1.0 / np.sqrt(deg)
    # stilde incl the appended self-loop; pads behave as isolated self-nodes
    stilde = np.ones(TR)
    stilde[:N] = np.bincount(col, weights=dinv[row], minlength=N)

    # layer-0 norm-weighted vocab counts u0[d, v] (structure-only prep)
    w_e = dinv[row] * dinv[col]
    u0 = np.bincount(col * V + x[row], weights=w_e,
                     minlength=N * V).reshape(N, V)
    u0p = np.zeros((TR, VP), dtype=np.float32)
    u0p[:N, :V] = u0

    order = _block_order()
    rank = np.empty(NGBLK, dtype=np.int64)
    rank[order] = np.arange(NGBLK)

    # ---- per-core edge prep (partition by SOURCE core) ----
    # only the E raw edges: the appended self-loops are handled on-device by
    # adding the local acheck table to the RS output (uhatT += aT)
    erow, ecol = row[:E], col[:E]
    csrc = erow // NDp
    src_local = erow % NDp
    tsrc = (src_local % P) * NBLK + src_local // P    # acheck table row
    B = (ecol // NDp) * NBLK + (ecol % NDp) // P      # global dst block
    rB = rank[B]                                      # ORDER position
    q = (ecol % NDp) % P                              # dst pos within block

    counts = np.zeros((NC, NGBLK), dtype=np.int64)
    per_core = []
    for c in range(NC):
        m = csrc == c
        t_c, r_c, q_c = tsrc[m], rB[m], q[m]
        o = np.argsort(r_c, kind="stable")
        t_c, q_c = t_c[o], q_c[o]
        bnd = np.searchsorted(r_c[o], np.arange(NGBLK + 1))
        counts[c] = bnd[1:] - bnd[:-1]
        per_core.append((t_c, q_c, bnd))

    cnt_eq = np.maximum(counts.max(axis=0), 1)        # shared per-pos count
    sched = tuple(int(v) for v in cnt_eq)
    SCH = _schedule(cnt_eq)

    ins = []
    for c in range(NC):
        t_c, q_c, bnd = per_core[c]
        idx_parts, cr_parts = [], []
        for gi, (pidx, cB, j0, gr, pos0) in enumerate(_granules()):
            kg, nsl, segs = SCH[gi]
            idx = np.zeros(kg * P, dtype=np.int64)
            crel = np.full(kg * P, -1.0, dtype=np.float32)
            used = [0] * gr
            for (k, jj, slot, p0, p1, first, last) in segs:
                pos = pos0 + jj
                n_c = bnd[pos + 1] - bnd[pos]
                r0 = k * P + p0
                m = p1 - p0
                u = used[jj]
                used[jj] += m
                if n_c == 0:
                    continue
                tseg = t_c[bnd[pos]:bnd[pos + 1]]
                qseg = q_c[bnd[pos]:bnd[pos + 1]]
                idx[r0:r0 + m] = tseg[min(u, n_c - 1)]
                nreal = min(n_c - u, m)
                if nreal > 0:
                    idx[r0:r0 + nreal] = tseg[u:u + nreal]
                    crel[r0:r0 + nreal] = qseg[u:u + nreal] + P * slot
            idx_parts.append(idx)
            cr_parts.append(crel.reshape(kg, P).T)
        idx16 = _wrap16(np.concatenate(idx_parts))
        colrel = np.concatenate(cr_parts, axis=1).astype(np.float32)

        nodes = np.arange(c * NDp, (c + 1) * NDp)
        ins.append(dict(
            idx16=idx16,                                   # [128, CT*8]
            colrel=colrel,                                 # [128, CT]
            u0T=np.ascontiguousarray(
                u0p[nodes].T.astype(np.float16)),          # [32, NDp]
            stilde_row=stilde[nodes].astype(
                np.float16).reshape(1, NDp),
            rdinv_row=np.sqrt(deg[nodes]).astype(
                np.float16).reshape(1, NDp),
            dinvpm=_pm(dinv[nodes].astype(np.float32)),
            dinv2pm=_pm((dinv[nodes] ** 2).astype(np.float32)),
            sqdegpm32=_pm(np.sqrt(deg[nodes]).astype(np.float32)),
            sqdegpm16=_pm(np.sqrt(deg[nodes]).astype(np.float16)),
            degpm16=_pm(deg[nodes].astype(np.float16)),
            batchpm=_pm(np.where(nodes < N, batch[np.minimum(nodes, N - 1)],
                                 -1.0).astype(np.float32)),
        ))

    cntraw = np.bincount(batch, minlength=G).astype(np.float32)
    invcnt = (1.0 / np.maximum(cntraw, 1.0)).astype(np.float32)
    return sched, ins, (cntraw.reshape(1, G), invcnt.reshape(1, G))


def _build(sched):
    cnt_eq = np.array(sched, dtype=np.int64)
    SCH = _schedule(cnt_eq)
    kgs = [kg for (kg, _, _) in SCH]
    CT = sum(kgs)
    kgoff = np.concatenate([[0], np.cumsum(kgs)])     # chunk offset per granule

    nc = bacc.Bacc("TRN2", target_bir_lowering=False, debug=False,
                   num_devices=NC)

    embed_t = nc.dram_tensor("embed", [V, H], F32, kind="ExternalInput")
    W_t = nc.dram_tensor("W", [L, H, H], F32, kind="ExternalInput")
    b_t = nc.dram_tensor("b", [L, H], F32, kind="ExternalInput")
    gamma_t = nc.dram_tensor("gamma", [L, H], F32, kind="ExternalInput")
    beta_t = nc.dram_tensor("beta", [L, H], F32, kind="ExternalInput")
    idx16_t = nc.dram_tensor("idx16", [128, CT * 8], I16, kind="ExternalInput")
    colrel_t = nc.dram_tensor("colrel", [128, CT], F32, kind="ExternalInput")
    u0T_t = nc.dram_tensor("u0T", [VP, NDp], F16, kind="ExternalInput")
    stilde_t = nc.dram_tensor("stilde_row", [1, NDp], F16, kind="ExternalInput")
    rdinv_t = nc.dram_tensor("rdinv_row", [1, NDp], F16, kind="ExternalInput")
    dinvpm_t = nc.dram_tensor("dinvpm", [128, NBLK], F32, kind="ExternalInput")
    dinv2pm_t = nc.dram_tensor("dinv2pm", [128, NBLK], F32, kind="ExternalInput")
    sqdegpm32_t = nc.dram_tensor("sqdegpm32", [128, NBLK], F32, kind="ExternalInput")
    sqdegpm16_t = nc.dram_tensor("sqdegpm16", [128, NBLK], F16, kind="ExternalInput")
    degpm16_t = nc.dram_tensor("degpm16", [128, NBLK], F16, kind="ExternalInput")
    batchpm_t = nc.dram_tensor("batchpm", [128, NBLK], F32, kind="ExternalInput")
    cntrow_t = nc.dram_tensor("cntrow", [1, G], F32, kind="ExternalInput")
    invcntrow_t = nc.dram_tensor("invcntrow", [1, G], F32, kind="ExternalInput")
    out_t = nc.dram_tensor("out", [G, H], F32, kind="ExternalOutput")

    rg = [list(range(NC))]
    AF = mybir.ActivationFunctionType
    OP = mybir.AluOpType
    GRAN = _granules()

    with tile.TileContext(nc) as tc:
        with tc.tile_pool(name="big", bufs=1) as big, \
             tc.tile_pool(name="sm", bufs=1) as sm, \
             tc.tile_pool(name="smd", bufs=2) as smd, \
             tc.tile_pool(name="gpool", bufs=3) as gpool, \
             tc.tile_pool(name="spool", bufs=2) as spool, \
             tc.tile_pool(name="scpool", bufs=6) as scpool, \
             tc.tile_pool(name="stg", bufs=2) as stgp, \
             tc.tile_pool(name="ps", bufs=2, space="PSUM") as ps, \
             tc.tile_pool(name="psacc", bufs=2, space="PSUM") as psacc, \
             tc.tile_pool(name="dram", bufs=1, space="DRAM") as dram:

            # ---------- constants / inputs ----------
            ident = sm.tile([P, P], F32)
            make_identity(nc, ident[:])
            ident16 = sm.tile([P, P], F16)
            nc.vector.tensor_copy(ident16[:], ident[:])
            iota_i = sm.tile([P, 2 * P], I32)
            nc.gpsimd.iota(iota_i[:], pattern=[[1, 2 * P]], base=0,
                           channel_multiplier=0)
            iota16 = sm.tile([P, 2 * P], F16)
            nc.vector.tensor_copy(iota16[:], iota_i[:])
            iota64_i = sm.tile([P, G], I32)
            nc.gpsimd.iota(iota64_i[:], pattern=[[1, G]], base=0,
                           channel_multiplier=0)
            iota64_f = sm.tile([P, G], F32)
            nc.vector.tensor_copy(iota64_f[:], iota64_i[:])

            # load order: L0's dependencies first (embed/W0/u0T/cols), the
            # big gather index tables after (first needed ~50us in)
            embsb = sm.tile([V, H], F32)
            nc.sync.dma_start(embsb[:], embed_t.ap())
            # u0T lives in the S-pool ring: only needed during layer 0, its
            # buffer is recycled afterwards
            u0T_sb = spool.tile([VP, NDp], F16, tag="S", name="u0T")
            nc.sync.dma_start(u0T_sb[:], u0T_t.ap())
            dinvpm_sb = sm.tile([128, NBLK], F32)
            nc.sync.dma_start(dinvpm_sb[:], dinvpm_t.ap())
            sqdeg16_sb = sm.tile([128, NBLK], F16)
            nc.sync.dma_start(sqdeg16_sb[:], sqdegpm16_t.ap())
            deg16_sb = sm.tile([128, NBLK], F16)
            nc.sync.dma_start(deg16_sb[:], degpm16_t.ap())

            Wsb = [sm.tile([H, H], F32, name=f"W{l}") for l in range(L)]
            bcol = [sm.tile([H, 1], F32, name=f"b{l}") for l in range(L)]
            gcol = [sm.tile([H, 1], F32, name=f"g{l}") for l in range(L)]
            betacol = [sm.tile([H, 1], F32, name=f"be{l}") for l in range(L)]
            for l in range(L):
                nc.sync.dma_start(Wsb[l][:], W_t.ap()[l])
                nc.sync.dma_start(bcol[l][:], b_t.ap()[l, :, None])
                nc.sync.dma_start(gcol[l][:], gamma_t.ap()[l, :, None])
                nc.sync.dma_start(betacol[l][:], beta_t.ap()[l, :, None])
            b32row = [sm.tile([1, H], F32, name=f"b32r{l}") for l in range(L)]
            b16row = [sm.tile([1, H], F16, name=f"b16r{l}") for l in range(L)]
            for l in range(L):
                nc.sync.dma_start(b32row[l][:], b_t.ap()[l:l + 1, :])
                nc.vector.tensor_copy(b16row[l][:], b32row[l][:])
            idx_sb = sm.tile([128, CT * 8], I16)
            nc.sync.dma_start(idx_sb[:], idx16_t.ap())
            colrel_sb = sm.tile([128, CT], F32)
            nc.sync.dma_start(colrel_sb[:], colrel_t.ap())
            stilde_sb = sm.tile([1, NDp], F16)
            nc.sync.dma_start(stilde_sb[:], stilde_t.ap())
            rdinv_sb = sm.tile([1, NDp], F16)
            nc.sync.dma_start(rdinv_sb[:], rdinv_t.ap())
            dinv2pm_sb = sm.tile([128, NBLK], F32)
            nc.sync.dma_start(dinv2pm_sb[:], dinv2pm_t.ap())
            sqdeg32_sb = sm.tile([128, NBLK], F32)
            nc.sync.dma_start(sqdeg32_sb[:], sqdegpm32_t.ap())
            batchpm_sb = sm.tile([128, NBLK], F32)
            nc.sync.dma_start(batchpm_sb[:], batchpm_t.ap())
            cntbc = sm.tile([128, G], F32)
            nc.sync.dma_start(cntbc[:], bass.AP(tensor=cntrow_t, offset=0,
                                                ap=[[0, 128], [1, G]]))
            invcntbc = sm.tile([128, G], F32)
            nc.sync.dma_start(invcntbc[:], bass.AP(tensor=invcntrow_t,
                                                   offset=0,
                                                   ap=[[0, 128], [1, G]]))
            eps_sb = sm.tile([H, 1], F32)
            nc.vector.memset(eps_sb[:], BN_EPS)
            ones_row = sm.tile([1, NDp], F16)
            nc.vector.memset(ones_row[:], 1.0)

            # ---------- persistent big tiles ----------
            A16 = [big.tile([128, NBLK, H], F16, name=f"A16_{i}")
                   for i in range(2)]
            uhatT = big.tile([128, NDp], F16)
            aT = big.tile([128, NDp], F16)      # transposed acheck (self-add)

            # ---------- DRAM scratch ----------
            atab_d = [dram.tile([NDp, H], F8, name=f"atab{l}")
                      for l in range(L - 1)]
            # piece-A slabs carry 2 extra rows: the previous layer's BN stat
            # sums ride the ReduceScatter (replicated per slab -> all-reduced)
            pagg_d = [[dram.tile(
                [NC * (SLABR if pi == 0 else 128), nj * P], F16,
                name=f"pagg{i}_{pi}") for pi, (_, nj) in enumerate(PIECES)]
                for i in range(2)]
            rsout_d = [[dram.tile(
                [(SLABR if pi == 0 else 128), nj * P], F16,
                name=f"rsout{i}_{pi}") for pi, (_, nj) in enumerate(PIECES)]
                for i in range(2)]
            pool_i = dram.tile([G + 2, H], F16)
            pool_o = dram.tile([NC, G + 2, H], F16, addr_space="Shared")

            # ---------- EW0 = (embed @ W0) as f16 [VP, H] ----------
            trp = ps.tile([P, P], F32, tag="tr", space="PSUM", bufs=1)
            nc.tensor.transpose(out=trp[:, 0:V], in_=embsb[:],
                                identity=ident[0:V, 0:V])
            embT = sm.tile([128, VP], F32)
            nc.vector.memset(embT[:], 0.0)
            nc.vector.tensor_copy(embT[:, 0:V], trp[:, 0:V])
            ew0p = ps.tile([P, H], F32, tag="mp", space="PSUM", bufs=2)
            nc.tensor.matmul(out=ew0p[0:VP, :], lhsT=embT[:], rhs=Wsb[0][:],
                             start=True, stop=True)
            EW0 = sm.tile([VP, H], F16)
            nc.vector.tensor_copy(EW0[:], ew0p[0:VP, :])

            poolp = psacc.tile([G, H], F32, tag="pool", space="PSUM",
                               bufs=1)

            def transform(l, j_lo, j_hi, lhsT_fn, rhs_main, extra_rank1,
                          scale_col_sb, SA, SSA):
                """blocks j_lo..j_hi-1: pre-psum -> acheck + stats (+pool).
                Two sweeps so the in-order PE queue never stalls on the
                DVE-produced acheck: (1) pre-matmuls + fused scale/relu,
                (2) stats/pool matmuls reading A16."""
                Ai = A16[l % 2]
                for j in range(j_lo, j_hi):
                    pre = ps.tile([P, H], F32, tag="mp", space="PSUM", bufs=2)
                    nterm = 1 + len(extra_rank1)
                    nc.tensor.matmul(out=pre[:], lhsT=lhsT_fn(j),
                                     rhs=rhs_main, start=True,
                                     stop=(nterm == 1))
                    for t, (rsb, rrow) in enumerate(extra_rank1):
                        nc.tensor.matmul(out=pre[:],
                                         lhsT=rsb[0:1, j * P:(j + 1) * P],
                                         rhs=rrow, start=False,
                                         stop=(t == nterm - 2))
                    # acheck = relu(scale * pre)  (fused mult + max 0)
                    nc.vector.tensor_scalar(
                        out=Ai[:, j, :], in0=pre[:],
                        scalar1=scale_col_sb[:, j:j + 1], scalar2=0.0,
                        op0=OP.mult, op1=OP.max)
                for j in range(j_lo, j_hi):
                    sq = smd.tile([P, H], F16, tag="sq", bufs=3)
                    nc.vector.tensor_mul(sq[:], Ai[:, j, :], Ai[:, j, :])
                    nc.tensor.matmul(out=SA, lhsT=sqdeg16_sb[:, j:j + 1],
                                     rhs=Ai[:, j, :], start=(j == 0),
                                     stop=(j == NBLK - 1))
                    nc.tensor.matmul(out=SSA, lhsT=deg16_sb[:, j:j + 1],
                                     rhs=sq[:], start=(j == 0),
                                     stop=(j == NBLK - 1))
                    if l == L - 1:
                        Sb = smd.tile([P, G], F16, tag="Sb", bufs=3)
                        nc.vector.tensor_scalar(
                            out=Sb[:], in0=iota64_f[:],
                            scalar1=batchpm_sb[:, j:j + 1],
                            scalar2=sqdeg32_sb[:, j:j + 1],
                            op0=OP.is_equal, op1=OP.mult)
                        nc.tensor.matmul(out=poolp[:], lhsT=Sb[:],
                                         rhs=Ai[:, j, :], start=(j == 0),
                                         stop=(j == NBLK - 1))

            def bn_cols(l, mvT, apad_col):
                """mvT [H, 2] f32 (AR'd sums) -> c1, c0 columns [H, 1].
                apad_col = value of a at pad nodes this layer (isolated-node
                recursion), used for the analytic pad correction."""
                # mvT rows arrive pre-scaled by 1/N (f16 overflow safety)
                rbs = smd.tile([H, 1], F32, tag="rbs")
                nc.vector.tensor_scalar(out=rbs[:], in0=apad_col[:],
                                        scalar1=float(NPAD) / N, scalar2=None,
                                        op0=OP.mult)
                mu = smd.tile([H, 1], F32, tag="mu")
                nc.vector.tensor_sub(mu[:], mvT[:, 0:1], rbs[:])
                rb2 = smd.tile([H, 1], F32, tag="rb2")
                nc.vector.tensor_mul(rb2[:], apad_col[:], apad_col[:])
                nc.vector.tensor_scalar(out=rb2[:], in0=rb2[:],
                                        scalar1=float(NPAD) / N, scalar2=None,
                                        op0=OP.mult)
                e2 = smd.tile([H, 1], F32, tag="e2")
                nc.vector.tensor_sub(e2[:], mvT[:, 1:2], rb2[:])
                var = smd.tile([H, 1], F32, tag="var")
                nc.vector.tensor_mul(var[:], mu[:], mu[:])
                nc.vector.tensor_sub(var[:], e2[:], var[:])
                sd = smd.tile([H, 1], F32, tag="sd")
                nc.scalar.activation(out=sd[:], in_=var[:], func=AF.Sqrt,
                                     bias=eps_sb[:], scale=1.0)
                rstd = smd.tile([H, 1], F32, tag="rstd")
                nc.vector.reciprocal(rstd[:], sd[:])
                c1 = smd.tile([H, 1], F32, tag=f"c1_{l}", bufs=1)
                nc.vector.tensor_mul(c1[:], gcol[l][:], rstd[:])
                c0 = smd.tile([H, 1], F32, tag=f"c0_{l}", bufs=1)
                nc.vector.tensor_mul(c0[:], mu[:], c1[:])
                nc.vector.tensor_sub(c0[:], betacol[l][:], c0[:])
                return c1, c0

            def emit_stats_ride(l, SA, SSA):
                """Write layer-l stat sums (f16) into every slab's stats rows
                of the NEXT layer's piece-A partial table; RS-A all-reduces
                them as a side effect."""
                sa16 = smd.tile([1, H], F16, tag="sasb")
                ss16 = smd.tile([1, H], F16, tag="sssb")
                nc.scalar.activation(out=sa16[:], in_=SA, func=AF.Copy,
                                     scale=1.0 / N)
                nc.scalar.activation(out=ss16[:], in_=SSA, func=AF.Copy,
                                     scale=1.0 / N)
                pa = pagg_d[l][0]
                for c in range(NC):
                    r0 = c * SLABR + 128
                    nc.sync.dma_start(pa[:][r0:r0 + 1, 0:H], sa16[:])
                    nc.sync.dma_start(pa[:][r0 + 1:r0 + 2, 0:H], ss16[:])

            def atab_write(l, j_lo, j_hi):
                src = A16[l % 2][:, j_lo:j_hi, :]
                dst = atab_d[l][:].rearrange(
                    "(p j) h -> p j h", p=128)[:, j_lo:j_hi, :]
                nc.sync.dma_start(dst, src)

            def transpose_some(l, js):
                """aT[:, j*128:(j+1)*128] = A16[l%2][:, j, :]^T (self-add).
                Transposes land in a 4-block PSUM strip; one ACT copy per
                strip."""
                js = list(js)
                for i0 in range(0, len(js), 4):
                    grp = js[i0:i0 + 4]
                    t16 = ps.tile([P, 4 * P], F16, tag="tr4", space="PSUM",
                                  bufs=2)
                    for i, j in enumerate(grp):
                        nc.tensor.transpose(out=t16[:, i * P:(i + 1) * P],
                                            in_=A16[l % 2][:, j, :],
                                            identity=ident16[:])
                    if all(grp[i] == grp[0] + i for i in range(len(grp))):
                        nc.scalar.activation(
                            out=aT[:, grp[0] * P:(grp[0] + len(grp)) * P],
                            in_=t16[:, 0:len(grp) * P], func=AF.Copy)
                    else:
                        for i, j in enumerate(grp):
                            nc.scalar.activation(
                                out=aT[:, j * P:(j + 1) * P],
                                in_=t16[:, i * P:(i + 1) * P], func=AF.Copy)

            apad = [sm.tile([H, 1], F32, name=f"apad{l}") for l in range(L)]
            nc.scalar.activation(out=apad[0][:], in_=bcol[0][:], func=AF.Relu)

            def apad_next(l, c1, c0):
                """apad[l] = relu((c1*apad[l-1] + c0) @ W[l] + b[l])."""
                hp = smd.tile([H, 1], F32, tag="hp")
                nc.vector.tensor_mul(hp[:], c1[:], apad[l - 1][:])
                nc.vector.tensor_add(hp[:], hp[:], c0[:])
                pcp = ps.tile([P, P], F32, tag="tr", space="PSUM", bufs=1)
                nc.tensor.matmul(out=pcp[0:H, 0:1], lhsT=Wsb[l][:],
                                 rhs=hp[:], start=True, stop=True)
                nc.scalar.activation(out=apad[l][:], in_=pcp[0:H, 0:1],
                                     func=AF.Relu, bias=bcol[l][:], scale=1.0)

            PH = {"l0": 0, "u1": 1, "t1": 2, "t2": 3, "s1": 4, "s2": 5,
                  "full": 9}[os.environ.get("KPHASE", "full")]

            def dump(ap):
                dbg = sm.tile([G, H], F32, name="dbg")
                nc.vector.tensor_copy(dbg[:], ap)
                nc.sync.dma_start(out_t.ap(), dbg[:])

            # ================= layer 0 =================
            SA0t = psacc.tile([1, H], F32, tag="SA", space="PSUM",
                             bufs=1)
            SSA0t = psacc.tile([1, H], F32, tag="SSA", space="PSUM",
                              bufs=1)
            SA0 = SA0t[:]
            SSA0 = SSA0t[:]
            transform(0, 0, NBLK, lambda j: u0T_sb[:, j * P:(j + 1) * P],
                      EW0[:], [(ones_row, b16row[0][:])], dinvpm_sb,
                      SA0, SSA0)
            atab_write(0, 0, NBLK)
            emit_stats_ride(0, SA0, SSA0)
            if PH == 0:
                dump(A16[0][0:G, 0, :])

            # ================= layers 1..2 =================
            SA = SSA = None
            NLAY = {0: 1, 1: 2, 2: 2, 3: L, 4: L, 5: L, 9: L}[PH]
            for l in range(1, NLAY):
                li = l - 1          # pagg/rsout/atab parity index

                def granules_of(pidx, pj0):
                    # distribute this piece's aT transposes (of layer l-1)
                    # across its granules so they hide inside the sweep and
                    # finish before this piece's readback self-add
                    nj_p = PIECES[pidx][1]
                    tq = list(range(pj0, pj0 + nj_p))
                    gi = 0
                    for gidx, (gp, cB, j0, gr, pos0) in enumerate(GRAN):
                        if gp != pidx:
                            continue
                        kg, nsl, segs = SCH[gidx]
                        o8 = int(kgoff[gidx]) * 8
                        oc = int(kgoff[gidx])
                        gt = gpool.tile([128, kg, H], F8, tag="gt")
                        nc.gpsimd.dma_gather(
                            out_ap=gt[:], in_ap=atab_d[l - 1][:],
                            idxs_ap=idx_sb[:, o8:o8 + kg * 8],
                            num_idxs=kg * P, num_idxs_reg=kg * P,
                            elem_size=H, single_packet=False, queue_num=0)
                        stg = stgp.tile([128, gr * P], F16, tag="stg")
                        slabr = SLABR if pidx == 0 else 128
                        Scur = [-1, None]
                        agg4 = [-1, None, 0]     # group idx, tile, width
                        def flush():
                            if agg4[1] is not None:
                                g4 = agg4[0]
                                nc.scalar.activation(
                                    out=stg[:, g4 * 4 * P:g4 * 4 * P + agg4[2]],
                                    in_=agg4[1][:], func=AF.Copy)
                        for (k, jj, slot, p0, p1, first, last) in segs:
                            if Scur[0] != k:
                                w2 = nsl[k] * P
                                S = scpool.tile([128, 2 * P], F16, tag="Sc")
                                nc.vector.tensor_scalar(
                                    out=S[:, 0:w2], in0=iota16[:, 0:w2],
                                    scalar1=colrel_sb[:, oc + k:oc + k + 1],
                                    scalar2=None, op0=OP.is_equal)
                                Scur = [k, S]
                            g4 = jj // 4
                            if g4 != agg4[0]:
                                flush()
                                w = min(4, gr - g4 * 4) * P
                                aggt = psacc.tile([H, w], F32, tag="agg",
                                                  bufs=2, name="agg4")
                                agg4 = [g4, aggt, w]
                            cc = (jj % 4) * P
                            nc.tensor.matmul(
                                out=agg4[1][:, cc:cc + P],
                                lhsT=gt[:, k, :],
                                rhs=Scur[1][:, slot * P:(slot + 1) * P],
                                start=first, stop=last)
                        flush()
                        dst = pagg_d[li][pidx][:][
                            cB * slabr:cB * slabr + 128,
                            0:gr * P]
                        nc.sync.dma_start(dst, stg[:])
                        gi += 1
                        hi = (nj_p * gi) // NC
                        lo = (nj_p * (gi - 1)) // NC
                        if hi > lo:
                            transpose_some(l - 1, tq[lo:hi])
                    nc.gpsimd.collective_compute(
                        "ReduceScatter", OP.add, replica_groups=rg,
                        ins=[pagg_d[li][pidx].opt()],
                        outs=[rsout_d[li][pidx].opt()])

                # piece A aggregation + RS first (stats rows ride along);
                # BN prep can then overlap the remaining pieces' aggregation
                granules_of(0, PIECES[0][0])

                # ---- BN coefficients from stats ridden on RS-A ----
                ars = smd.tile([2, H], F16, tag="ars")
                nc.sync.dma_start(ars[:], rsout_d[li][0][:][128:130, 0:H])
                ars32 = smd.tile([2, H], F32, tag="ars32")
                nc.vector.tensor_copy(ars32[:], ars[:])
                mvp = ps.tile([P, P], F32, tag="tr", space="PSUM", bufs=1)
                nc.tensor.transpose(out=mvp[0:H, 0:2], in_=ars32[:],
                                    identity=ident[0:2, 0:2])
                mvT = smd.tile([H, 2], F32, tag="mvT")
                nc.vector.tensor_copy(mvT[:], mvp[0:H, 0:2])
                c1, c0 = bn_cols(l - 1, mvT, apad[l - 1])
                apad_next(l, c1, c0)
                Wc1 = smd.tile([H, H], F16, tag="Wc1")
                nc.vector.tensor_scalar(out=Wc1[:], in0=Wsb[l][:],
                                        scalar1=c1[:], scalar2=None,
                                        op0=OP.mult)
                c0wp = ps.tile([P, P], F32, tag="tr", space="PSUM", bufs=1)
                nc.tensor.matmul(out=c0wp[0:1, 0:H], lhsT=c0[:],
                                 rhs=Wsb[l][:], start=True, stop=True)
                c0Wrow = smd.tile([1, H], F16, tag="c0W")
                nc.vector.tensor_copy(c0Wrow[:], c0wp[0:1, 0:H])

                # remaining pieces' aggregation + RS
                for pidx in range(1, len(PIECES)):
                    granules_of(pidx, PIECES[pidx][0])

                # ---- per-piece readback + transform (+ table write) ----
                if PH != 1:
                    SAt = psacc.tile([1, H], F32, tag="SA", space="PSUM",
                                     bufs=1)
                    SSAt = psacc.tile([1, H], F32, tag="SSA", space="PSUM",
                                      bufs=1)
                    SA = SAt[:]
                    SSA = SSAt[:]
                for pidx, (pj0, nj) in enumerate(PIECES):
                    nc.sync.dma_start(
                        uhatT[:, pj0 * P:(pj0 + nj) * P],
                        rsout_d[li][pidx][:][0:128, :])
                    # self-loop contribution: uhat += acheck(l-1) transposed
                    nc.vector.tensor_add(
                        uhatT[:, pj0 * P:(pj0 + nj) * P],
                        uhatT[:, pj0 * P:(pj0 + nj) * P],
                        aT[:, pj0 * P:(pj0 + nj) * P])
                    if PH == 1:
                        continue
                    transform(l, pj0, pj0 + nj,
                              lambda j: uhatT[:, j * P:(j + 1) * P], Wc1[:],
                              [(stilde_sb, c0Wrow[:]),
                               (rdinv_sb, b16row[l][:])],
                              dinv2pm_sb, SA, SSA)
                    if l < L - 1:
                        atab_write(l, pj0, pj0 + nj)
                if PH == 1:
                    dump(uhatT[0:G, 0:H])
                elif PH == 2 and l == 1:
                    dj = int(os.environ.get("KDUMPJ", "0"))
                    dump(A16[1][0:G, dj, :])
                elif PH == 3 and l == L - 1:
                    dump(A16[(L - 1) % 2][0:G, 0, :])
                if PH in (3, 4, 5, 9) and l < L - 1:
                    emit_stats_ride(l, SA, SSA)

            if PH == 4:
                dump(poolp[:])
            if PH == 5:
                dbg5 = sm.tile([G, H], F32, name="dbg5")
                nc.vector.memset(dbg5[:], 0.0)
                nc.scalar.activation(out=dbg5[0:1, :], in_=SA, func=AF.Copy,
                                     scale=1.0 / N)
                nc.scalar.activation(out=dbg5[32:33, :], in_=SSA,
                                     func=AF.Copy, scale=1.0 / N)
                nc.sync.dma_start(out_t.ap(), dbg5[:])
            if PH == 9:
                # ================= pooling tail =================
                packp = sm.tile([G, H], F16)
                nc.vector.tensor_copy(packp[:], poolp[:])
                sa_sb = smd.tile([1, H], F16, tag="sasb")
                ss_sb = smd.tile([1, H], F16, tag="sssb")
                nc.scalar.activation(out=sa_sb[:], in_=SA, func=AF.Copy,
                                     scale=1.0 / N)
                nc.scalar.activation(out=ss_sb[:], in_=SSA, func=AF.Copy,
                                     scale=1.0 / N)
                nc.sync.dma_start(pool_i[:][0:G, :], packp[:])
                nc.sync.dma_start(pool_i[:][G:G + 1, :], sa_sb[:])
                nc.sync.dma_start(pool_i[:][G + 1:G + 2, :], ss_sb[:])
                nc.gpsimd.collective_compute(
                    "AllGather", OP.bypass, replica_groups=rg,
                    ins=[pool_i.opt()], outs=[pool_o.opt()])
                parf = sm.tile([G + 2, NC * H], F16)
                nc.sync.dma_start(
                    parf[:], bass.AP(tensor=pool_o.tensor,
                                     offset=pool_o[:].offset,
                                     ap=[[H, G + 2], [(G + 2) * H, NC], [1, H]]))
                for s in (4, 2, 1):
                    for k in range(s):
                        nc.vector.tensor_add(
                            parf[:, k * H:(k + 1) * H],
                            parf[:, k * H:(k + 1) * H],
                            parf[:, (k + s) * H:(k + s + 1) * H])
                ptp = ps.tile([P, P], F16, tag="tr", space="PSUM", bufs=1)
                nc.tensor.transpose(out=ptp[0:H, 0:G + 2], in_=parf[:, 0:H],
                                    identity=ident16[0:G + 2, 0:G + 2])
                parT = sm.tile([H, G + 2], F32)
                nc.vector.tensor_copy(parT[:], ptp[0:H, 0:G + 2])
                c1f, c0f = bn_cols(L - 1, parT[:, G:G + 2], apad[L - 1])
                # outT[c,g] = (c1[c]*poolT + c0[c]*cnt[g]) * invcnt[g]
                t2 = sm.tile([H, G], F32)
                nc.vector.tensor_tensor(out=t2[:], in0=parT[:, 0:G],
                                        in1=invcntbc[:], op=OP.mult)
                nc.vector.tensor_scalar(out=t2[:], in0=t2[:], scalar1=c1f[:],
                                        scalar2=c0f[:], op0=OP.mult,
                                        op1=OP.add)
                fint = ps.tile([P, P], F32, tag="tr", space="PSUM", bufs=1)
                nc.tensor.transpose(out=fint[:G, :], in_=t2[:], identity=ident[:])
                outsb = sm.tile([G, H], F32)
                nc.vector.tensor_copy(outsb[:], fint[:G, :])
                nc.sync.dma_start(out_t.ap(), outsb[:])

    nc.compile()
    return nc


_NC_CACHE = {}


def _get_nc(sched):
    key = (sched, os.environ.get("KPHASE", "full"))
    if key not in _NC_CACHE:
        _NC_CACHE[key] = _build(sched)
    return _NC_CACHE[key]


def run(x, edge_index, batch, embed, W, b, gamma, beta, trace=False):
    sched, per_core, (cntrow, invcntrow) = _prep(x, edge_index, batch)
    nc = _get_nc(sched)
    shared = dict(
        embed=np.ascontiguousarray(np.asarray(embed, dtype=np.float32)),
        W=np.ascontiguousarray(np.asarray(W, dtype=np.float32)),
        b=np.ascontiguousarray(np.asarray(b, dtype=np.float32)),
        gamma=np.ascontiguousarray(np.asarray(gamma, dtype=np.float32)),
        beta=np.ascontiguousarray(np.asarray(beta, dtype=np.float32)),
        cntrow=cntrow,
        invcntrow=invcntrow,
    )
    in_maps = [{**shared, **per_core[c]} for c in range(NC)]
    res = run_bass_kernel_spmd(nc, in_maps, core_ids=list(range(NC)),
                               trace=trace)
    return res.results[0]["out"], res


def kernel(x, edge_index, batch, embed, W, b, gamma, beta):
    out, _ = run(x, edge_index, batch, embed, W, b, gamma, beta)
    return out
